# revision 1
# baseline (speedup 1.0000x reference)
"""Trainium2 Bass kernel for the exact-match memorizer lookup.

Dense PE brute force, queries sharded 512/core, memory replicated.

Row order everywhere is j = a*8192 + c*32 + pp  (a,pp = partition p = a*32+pp,
c = free column), so the DVE 32x32 block transpose produces contraction-major
operands whose column order equals j.

Per core (512 queries as 4 groups g of 128; query q = g*128 + m):
  match(q, j)  <=>  dist2(x_q, k_j) == 0   (features are ints 0..3)
  One matmul per tile computes, exactly, in f32 PSUM:
      psum[m, j] = ||k_j||^2 - 2 x_q . k_j + ramp_j * 2^-11
  via an augmented 32-row bf16 contraction (keys, ||k||^2, ramp, zeros),
  with ramp_j = 128 - (j & 127) in [1, 128].
  ACT adds ||x_q||^2 (per-partition bias, Relu = identity since t > 0):
  matches give t = ramp*2^-11 <= 2^-4, non-matches t >= ~1.
  A segmented reduce_min over 128-column blocks gives per (q, block) the
  LAST matching j in the block (min ramp = largest j), else ~>=1.
  A small pass over the 256 block minima picks the last matching block and
  decodes j*.  Gather mem_values[j*] (rows padded to 64B), select vs the
  linear fallback x @ w.T + b.
"""

import sys

if "/opt/trn_rl_repo" not in sys.path:
    sys.path.insert(0, "/opt/trn_rl_repo")

import numpy as np

import bass_rust
from concourse.bass import Bass, IndirectOffsetOnAxis
import concourse.tile as tile
from concourse import bass, mybir

N_QUERIES = 4096
N_MEM = 32768
D_FEAT = 8
N_CORES = 8
NQ = N_QUERIES // N_CORES  # 512 queries per core
QS = NQ // 128  # 4 query groups per core
MC = N_MEM // 128  # 256 memory rows per partition
KAUG = 32  # augmented contraction dim (bf16, padded for 32x32 transpose)
TW = 2048  # memory rows per fused PSUM tile (4 banks)
NTILE = N_MEM // TW  # 16
BLK = 128  # reduce segment (= ramp period)
NBLK = N_MEM // BLK  # 256
RS = 2.0 ** -11  # ramp scale

F32 = mybir.dt.float32
BF16 = mybir.dt.bfloat16
I32 = mybir.dt.int32
U8 = mybir.dt.uint8


def _patch_tile_drain():
    """This container's walrus accepts only one sync-wait per instruction;
    TileContext's teardown drain waits on every used semaphore at once.
    Split it into one drain per semaphore."""
    if getattr(tile.TileContext, "_drain_patched", False):
        return
    from concourse.tile import ScopedClock

    def _drain_and_barrier(self, tick_clock, wait_clock):
        gc = tick_clock.global_clock
        ticks = eval(repr(gc).replace("VectorClock(", "").rstrip(")"))
        for i, t in enumerate(ticks):
            if t <= 0:
                continue
            part = [t if j == i else 0 for j in range(len(ticks))]
            d = self.nc.sync.drain()
            wait_clock.add_sem_waits(
                d.ins, ScopedClock({None: bass_rust.VectorClock(part)})
            )
        self.nc.all_engine_barrier()
        assert self.sems is not None
        popped = self.nc._tile_sem_poison_stack.pop()
        assert popped is self._sem_poison
        self.nc.clear_and_free_semaphores(list(self.sems.allocated().values()))
        self.nc.all_engine_barrier()

    tile.TileContext._drain_and_barrier = _drain_and_barrier
    tile.TileContext._drain_patched = True


def _fix_multiwaits(bir_bytes: bytes) -> bytes:
    """Hoist extra sync-waits onto standalone EventSemaphore instructions
    inserted immediately before the offender (same engine => identical
    in-order blocking semantics)."""
    import json

    bir = json.loads(bir_bytes)
    for f in bir["functions"]:
        for blk in f["blocks"]:
            insts = blk["instructions"]
            out_insts = []
            changed = False
            for inst in insts:
                si = inst.get("sync_info")
                waits = si.get("on_wait", []) if si else []
                if len(waits) > 1:
                    changed = True
                    for k, wv in enumerate(waits[:-1]):
                        out_insts.append(
                            {
                                "debug": inst.get("debug", 0),
                                "engine": inst["engine"],
                                "ins": [],
                                "name": f"{inst['name']}-sw{k}",
                                "opcode": "EventSemaphore",
                                "outs": [],
                                "sync_info": {"on_update": [], "on_wait": [wv]},
                            }
                        )
                    si["on_wait"] = [waits[-1]]
                out_insts.append(inst)
            if changed:
                blk["instructions"] = out_insts
    return json.dumps(bir).encode()


def build_nc(debug: bool = False) -> Bass:
    _patch_tile_drain()
    nc = Bass()
    AX = mybir.AxisListType
    OP = mybir.AluOpType

    x = nc.dram_tensor("x", [NQ, D_FEAT], F32, kind="ExternalInput")
    xa = nc.dram_tensor("xa", [NQ, D_FEAT], F32, kind="ExternalInput")
    mem_keys = nc.dram_tensor("mem_keys", [N_MEM, D_FEAT], F32, kind="ExternalInput")
    mem_values = nc.dram_tensor("mem_values", [N_MEM], F32, kind="ExternalInput")
    w = nc.dram_tensor("w", [1, D_FEAT], F32, kind="ExternalInput")
    b = nc.dram_tensor("b", [1], F32, kind="ExternalInput")
    out = nc.dram_tensor("out", [NQ, 1], F32, kind="ExternalOutput")

    mvpad_d = nc.dram_tensor("mvpad_d", [N_MEM, 16], F32)  # 64B value rows

    with tile.TileContext(nc) as tc:
        with (
            tc.tile_pool(name="sbuf", bufs=1) as pool,
            tc.tile_pool(name="work", bufs=8) as wpool,
            tc.tile_pool(name="psum", bufs=2, space="PSUM") as ppool,
        ):
            # ---- loads (row order j = a*8192 + c*32 + pp) -------------------
            keys_t = pool.tile([128, MC * D_FEAT], F32, tag="keys")
            nc.sync.dma_start(
                out=keys_t[:].rearrange("p (c d) -> p c d", d=D_FEAT),
                in_=mem_keys[:].rearrange("(p c) d -> p c d", p=128),
            )
            keys_v = keys_t[:].rearrange("p (c d) -> p c d", d=D_FEAT)

            vals_t = pool.tile([128, MC], F32, tag="vals")
            nc.sync.dma_start(
                out=vals_t[:].rearrange("p (c one) -> p c one", one=1),
                in_=mem_values[:, None].rearrange("(p c) one -> p c one", p=128),
            )

            # x in layout A (for the transpose): row q = a*128 + s*32 + pp
            xqa_t = pool.tile([128, QS * D_FEAT], F32, tag="xqa")
            nc.sync.dma_start(
                out=xqa_t[:].rearrange("p (s d) -> p s d", d=D_FEAT),
                in_=xa[:].rearrange("(p s) d -> p s d", p=128),
            )
            xqa_v = xqa_t[:].rearrange("p (s d) -> p s d", d=D_FEAT)

            # x in layout B (for everything else): row q = g*128 + m
            xqb_t = pool.tile([128, QS * D_FEAT], F32, tag="xqb")
            nc.sync.dma_start(
                out=xqb_t[:].rearrange("p (g d) -> p g d", d=D_FEAT),
                in_=x[:].rearrange("(g m) d -> m g d", m=128),
            )
            xqb_v = xqb_t[:].rearrange("p (g d) -> p g d", d=D_FEAT)

            w_t = pool.tile([128, D_FEAT], F32, tag="wt")
            nc.sync.dma_start(out=w_t[:], in_=w[0:1, :].to_broadcast([128, D_FEAT]))
            b_t = pool.tile([128, 1], F32, tag="bt")
            nc.sync.dma_start(out=b_t[:], in_=b[None, :].to_broadcast([128, 1]))

            # ---- padded value rows (64B) for the row-gather -----------------
            vpad_t = pool.tile([128, MC * 16], F32, tag="vpad")
            nc.vector.tensor_copy(
                out=vpad_t[:].rearrange("p (c e) -> p c e", e=16),
                in_=vals_t[:, :, None].to_broadcast([128, MC, 16]),
            )
            st_mv = nc.sync.dma_start(
                out=mvpad_d[:].rearrange("(p c) e -> p c e", p=128),
                in_=vpad_t[:].rearrange("p (c e) -> p c e", e=16),
            )

            # ---- key staging [p, c, 32] bf16: k_d..., |k|^2, ramp, 0... -----
            kpad_t = pool.tile([128, MC * KAUG], BF16, tag="kpad")
            kpad_v = kpad_t[:].rearrange("p (c e) -> p c e", e=KAUG)
            nc.vector.memset(kpad_t[:], 0)
            nc.vector.tensor_copy(out=kpad_v[:, :, 0:D_FEAT], in_=keys_v)
            ksq_t = pool.tile([128, MC * D_FEAT], F32, tag="ksq")
            nc.vector.tensor_tensor(
                out=ksq_t[:].rearrange("p (c d) -> p c d", d=D_FEAT),
                in0=keys_v, in1=keys_v, op=OP.mult,
            )
            knorm_t = pool.tile([128, MC], F32, tag="knorm")
            nc.vector.reduce_sum(
                out=knorm_t[:],
                in_=ksq_t[:].rearrange("p (c d) -> p c d", d=D_FEAT),
                axis=AX.X,
            )
            nc.vector.tensor_copy(out=kpad_v[:, :, 8:9], in_=knorm_t[:, :, None])
            # ramp = (128 - i)*2^-11,  i = j & 127 = (c & 3)*32 + (p & 31)
            pp_t = pool.tile([128, 1], I32, tag="ppi")
            nc.gpsimd.iota(pp_t[:], [[1, 1]], base=0, channel_multiplier=1)
            nc.vector.tensor_scalar(
                out=pp_t[:], in0=pp_t[:], scalar1=31, scalar2=None,
                op0=OP.bitwise_and,
            )
            ppf_t = pool.tile([128, 1], F32, tag="ppf")
            nc.vector.tensor_copy(out=ppf_t[:], in_=pp_t[:])
            ci_t = pool.tile([128, MC], I32, tag="ci")
            nc.gpsimd.iota(ci_t[:], [[1, MC]], base=0, channel_multiplier=0)
            nc.vector.tensor_scalar(
                out=ci_t[:], in0=ci_t[:], scalar1=3, scalar2=None,
                op0=OP.bitwise_and,
            )
            ramp_t = pool.tile([128, MC], F32, tag="ramp")
            nc.vector.tensor_copy(out=ramp_t[:], in_=ci_t[:])
            nc.vector.tensor_scalar(
                out=ramp_t[:], in0=ramp_t[:], scalar1=32.0, scalar2=None,
                op0=OP.mult,
            )
            nc.vector.tensor_scalar_add(ramp_t[:], ramp_t[:], ppf_t[:, 0:1])
            nc.vector.tensor_scalar(
                out=ramp_t[:], in0=ramp_t[:], scalar1=128.0, scalar2=-RS,
                op0=OP.subtract, op1=OP.mult,
            )  # (i - 128) * -2^-11
            nc.vector.tensor_copy(out=kpad_v[:, :, 9:10], in_=ramp_t[:, :, None])

            # ---- query staging [p, s, 32] bf16: -2 x_d..., 1, 1, 0... -------
            xpad_t = pool.tile([128, QS * KAUG], BF16, tag="xpad")
            xpad_v = xpad_t[:].rearrange("p (s e) -> p s e", e=KAUG)
            nc.vector.memset(xpad_t[:], 0)
            xm2_t = pool.tile([128, QS * D_FEAT], F32, tag="xm2")
            nc.vector.tensor_scalar_mul(xm2_t[:], xqa_t[:], -2.0)
            nc.vector.tensor_copy(
                out=xpad_v[:, :, 0:D_FEAT],
                in_=xm2_t[:].rearrange("p (s d) -> p s d", d=D_FEAT),
            )
            one_t = pool.tile([128, QS], F32, tag="onef")
            nc.vector.memset(one_t[:], 1.0)
            nc.vector.tensor_copy(out=xpad_v[:, :, 8:9], in_=one_t[:, :, None])
            nc.vector.tensor_copy(out=xpad_v[:, :, 9:10], in_=one_t[:, :, None])

            # ---- on-chip 32x32 block transposes -----------------------------
            ktr = []
            for a in range(4):
                kt = pool.tile([32, MC * KAUG], BF16, name=f"ktr{a}", tag=f"ktr{a}")
                # shape [32, 8192]
                nc.vector.transpose(
                    out=kt[:, : MC * KAUG], in_=kpad_t[32 * a : 32 * (a + 1), :]
                )
                ktr.append(kt)
            xlh = []
            for g in range(4):
                xt = pool.tile([32, 128], BF16, name=f"xlh{g}", tag=f"xlh{g}")
                nc.vector.transpose(
                    out=xt[:], in_=xpad_t[32 * g : 32 * (g + 1), :]
                )
                xlh.append(xt)

            # ---- xnorm + linear fallback (layout B) -------------------------
            xsq_t = pool.tile([128, QS * D_FEAT], F32, tag="xsq")
            nc.vector.tensor_tensor(
                out=xsq_t[:].rearrange("p (g d) -> p g d", d=D_FEAT),
                in0=xqb_v, in1=xqb_v, op=OP.mult,
            )
            xnorm_t = pool.tile([128, QS], F32, tag="xnorm")
            nc.vector.reduce_sum(
                out=xnorm_t[:],
                in_=xsq_t[:].rearrange("p (g d) -> p g d", d=D_FEAT),
                axis=AX.X,
            )
            xw_t = pool.tile([128, QS * D_FEAT], F32, tag="xw")
            nc.vector.tensor_tensor(
                out=xw_t[:].rearrange("p (g d) -> p g d", d=D_FEAT),
                in0=xqb_v,
                in1=w_t[:, None, :].to_broadcast([128, QS, D_FEAT]),
                op=OP.mult,
            )
            linq_t = pool.tile([128, QS], F32, tag="linq")
            nc.vector.reduce_sum(
                out=linq_t[:],
                in_=xw_t[:].rearrange("p (g d) -> p g d", d=D_FEAT),
                axis=AX.X,
            )
            nc.vector.tensor_scalar_add(linq_t[:], linq_t[:], b_t[:, 0:1])

            # ---- main loop: matmul -> ACT bias/cast -> segmented min --------
            lastmin = [
                pool.tile([128, NBLK], F32, name=f"lastmin{g}", tag=f"lastmin{g}") for g in range(QS)
            ]
            for g in range(QS):
                for mt in range(NTILE):
                    a = mt // 4
                    off = (mt % 4) * TW
                    ps = ppool.tile([128, TW], F32, tag="ps")
                    for k in range(TW // 512):
                        nc.tensor.matmul(
                            out=ps[:, k * 512 : (k + 1) * 512],
                            lhsT=xlh[g][:, :],
                            rhs=ktr[a][:, off + k * 512 : off + (k + 1) * 512],
                            start=True,
                            stop=True,
                        )
                    tt = wpool.tile([128, TW], BF16, tag="tt")
                    nc.scalar.activation(
                        out=tt[:],
                        in_=ps[:],
                        func=mybir.ActivationFunctionType.Relu,
                        bias=xnorm_t[:, g : g + 1],
                        scale=1.0,
                    )
                    nc.vector.tensor_reduce(
                        out=lastmin[g][:, mt * (TW // BLK) : (mt + 1) * (TW // BLK)],
                        in_=tt[:].rearrange("p (n i) -> p n i", i=BLK),
                        axis=AX.X,
                        op=OP.min,
                    )

            # ---- pick last matching block, decode j* ------------------------
            skbase_t = pool.tile([128, NBLK], F32, tag="skbase")
            bi_t = pool.tile([128, NBLK], I32, tag="bi")
            nc.gpsimd.iota(bi_t[:], [[1, NBLK]], base=0, channel_multiplier=0)
            nc.vector.tensor_copy(out=skbase_t[:], in_=bi_t[:])
            nc.vector.tensor_scalar(
                out=skbase_t[:], in0=skbase_t[:], scalar1=256.0, scalar2=384.0,
                op0=OP.mult, op1=OP.add,
            )  # b*256 + 384

            kk_t = pool.tile([128, QS], F32, tag="kk")
            for g in range(QS):
                ism = wpool.tile([128, NBLK], F32, tag="ism")
                nc.vector.tensor_scalar(
                    out=ism[:], in0=lastmin[g][:], scalar1=0.5, scalar2=None,
                    op0=OP.is_lt,
                )
                sk = wpool.tile([128, NBLK], F32, tag="sk")
                nc.vector.tensor_scalar_mul(sk[:], lastmin[g][:], 2048.0)
                nc.vector.tensor_tensor(
                    out=sk[:], in0=skbase_t[:], in1=sk[:], op=OP.subtract
                )  # b*256 + 256 + i  (i = j & 127 of the last match)
                nc.vector.tensor_tensor(out=sk[:], in0=sk[:], in1=ism[:], op=OP.mult)
                nc.vector.reduce_max(out=kk_t[:, g : g + 1], in_=sk[:], axis=AX.X)

            found_t = pool.tile([128, QS], F32, tag="found")
            nc.vector.tensor_scalar(
                out=found_t[:], in0=kk_t[:], scalar1=0.0, scalar2=None, op0=OP.is_gt
            )
            # v = K - 256 = b*256 + i ; clamp >= 0 ; j* = (v>>8)*128 + (v&255)
            vv_t = pool.tile([128, QS], F32, tag="vv")
            nc.vector.tensor_scalar(
                out=vv_t[:], in0=kk_t[:], scalar1=-256.0, scalar2=0.0,
                op0=OP.add, op1=OP.max,
            )
            vi_t = pool.tile([128, QS], I32, tag="vi")
            nc.vector.tensor_copy(out=vi_t[:], in_=vv_t[:])
            bhi_t = pool.tile([128, QS], I32, tag="bhi")
            nc.vector.tensor_scalar(
                out=bhi_t[:], in0=vi_t[:], scalar1=8, scalar2=None,
                op0=OP.arith_shift_right,
            )
            nc.vector.tensor_scalar(
                out=bhi_t[:], in0=bhi_t[:], scalar1=7, scalar2=None,
                op0=OP.logical_shift_left,
            )
            plo_t = pool.tile([128, QS], I32, tag="plo")
            nc.vector.tensor_scalar(
                out=plo_t[:], in0=vi_t[:], scalar1=255, scalar2=None,
                op0=OP.bitwise_and,
            )
            jstar_t = pool.tile([128, QS], I32, tag="jstar")
            nc.vector.tensor_tensor(
                out=jstar_t[:], in0=bhi_t[:], in1=plo_t[:], op=OP.add
            )
            # remap n -> dram row r of mvpad_d (loaded in plain (p,c) order):
            # n = a*8192 + c*32 + pp  ->  r = a*8192 + pp*256 + c
            ra_t = pool.tile([128, QS], I32, tag="ra")
            nc.vector.tensor_scalar(
                out=ra_t[:], in0=jstar_t[:], scalar1=13, scalar2=13,
                op0=OP.arith_shift_right, op1=OP.logical_shift_left,
            )
            rb_t = pool.tile([128, QS], I32, tag="rb")
            nc.vector.tensor_scalar(
                out=rb_t[:], in0=jstar_t[:], scalar1=31, scalar2=8,
                op0=OP.bitwise_and, op1=OP.logical_shift_left,
            )
            rc_t = pool.tile([128, QS], I32, tag="rc")
            nc.vector.tensor_scalar(
                out=rc_t[:], in0=jstar_t[:], scalar1=5, scalar2=255,
                op0=OP.arith_shift_right, op1=OP.bitwise_and,
            )
            nc.vector.tensor_tensor(out=ra_t[:], in0=ra_t[:], in1=rb_t[:], op=OP.add)
            nc.vector.tensor_tensor(out=ra_t[:], in0=ra_t[:], in1=rc_t[:], op=OP.add)

            # ---- value gather (64B rows), select, store ---------------------
            val_t = pool.tile([128, QS], F32, tag="val")
            for g in range(QS):
                rg = pool.tile([128, 1], I32, name=f"rg{g}", tag=f"rg{g}")
                nc.vector.tensor_copy(out=rg[:], in_=ra_t[:, g : g + 1])
                vg = pool.tile([128, 16], F32, name=f"vg{g}", tag=f"vg{g}")
                gv = nc.gpsimd.indirect_dma_start(
                    out=vg[:],
                    out_offset=None,
                    in_=mvpad_d[:],
                    in_offset=IndirectOffsetOnAxis(ap=rg[:, 0:1], axis=0),
                )
                tile.add_dep_helper(gv.ins, st_mv.ins, reason="gather after pad")
                nc.vector.tensor_copy(out=val_t[:, g : g + 1], in_=vg[:, 0:1])

            found_u8 = pool.tile([128, QS], U8, tag="found_u8")
            nc.vector.tensor_copy(out=found_u8[:], in_=found_t[:])
            res_t = pool.tile([128, QS], F32, tag="res")
            nc.vector.select(
                out=res_t[:],
                mask=found_u8[:],
                on_true=val_t[:],
                on_false=linq_t[:],
            )
            nc.sync.dma_start(
                out=out[:].rearrange("(g m) one -> m g one", m=128),
                in_=res_t[:, :, None],
            )

            if debug:
                taps = {
                    "d_lastmin0": lastmin[0],
                    "d_kk": kk_t,
                    "d_found": found_t,
                    "d_vv": vv_t,
                    "d_linq": linq_t,
                    "d_xnorm": xnorm_t,
                }
                for name, t in taps.items():
                    shp = list(t[:].shape)
                    dt_ = nc.dram_tensor(name, shp, F32, kind="ExternalOutput")
                    nc.sync.dma_start(out=dt_[:], in_=t[:])
                for name, t, wdt in [
                    ("d_ktr0", ktr[0], BF16),
                    ("d_xlh0", xlh[0], BF16),
                    ("d_jstar", jstar_t, I32),
                    ("d_ra", ra_t, I32),
                    ("d_val", val_t, F32),
                ]:
                    shp = list(t[:].shape)
                    ft = pool.tile(shp, F32, name="tap_" + name, tag="tap_" + name)
                    nc.vector.tensor_copy(out=ft[:], in_=t[:])
                    dt_ = nc.dram_tensor(name, shp, F32, kind="ExternalOutput")
                    nc.sync.dma_start(out=dt_[:], in_=ft[:])

    return nc


_NC_CACHE: dict[str, Bass] = {}


def _get_nc() -> Bass:
    if "nc" not in _NC_CACHE:
        nc = build_nc()
        orig = nc.to_json_bytes
        nc.to_json_bytes = lambda: _fix_multiwaits(orig())
        _NC_CACHE["nc"] = nc
    return _NC_CACHE["nc"]


def kernel(x, mem_keys, mem_values, w, b):
    from concourse.bass_utils import run_bass_kernel_spmd

    x = np.ascontiguousarray(np.asarray(x, dtype=np.float32))
    mem_keys = np.ascontiguousarray(np.asarray(mem_keys, dtype=np.float32))
    mem_values = np.ascontiguousarray(np.asarray(mem_values, dtype=np.float32))
    w = np.ascontiguousarray(np.asarray(w, dtype=np.float32))
    b = np.ascontiguousarray(np.asarray(b, dtype=np.float32))

    nc = _get_nc()
    # device row order j = a*8192 + c*32 + pp maps to plain (p=a*32+pp, c) loads
    mk_perm = np.ascontiguousarray(
        mem_keys.reshape(4, 256, 32, D_FEAT).transpose(0, 2, 1, 3).reshape(N_MEM, D_FEAT)
    )
    mv_perm = np.ascontiguousarray(
        mem_values.reshape(4, 256, 32).transpose(0, 2, 1).reshape(N_MEM)
    )
    in_maps = [
        {
            "x": x[c * NQ : (c + 1) * NQ],
            "xa": np.ascontiguousarray(
                x[c * NQ : (c + 1) * NQ]
                .reshape(4, 4, 32, D_FEAT)
                .transpose(0, 2, 1, 3)
                .reshape(NQ, D_FEAT)
            ),
            "mem_keys": mk_perm,
            "mem_values": mv_perm,
            "w": w,
            "b": b,
        }
        for c in range(N_CORES)
    ]
    res = run_bass_kernel_spmd(nc, in_maps, core_ids=list(range(N_CORES)))
    return np.concatenate([r["out"] for r in res.results], axis=0)


if __name__ == "__main__":
    rng = np.random.default_rng(0)
    mk = rng.integers(0, 4, (N_MEM, D_FEAT)).astype(np.float32)
    xx = rng.integers(0, 4, (N_QUERIES, D_FEAT)).astype(np.float32)
    mv = rng.normal(size=N_MEM).astype(np.float32)
    ww = rng.normal(size=(1, D_FEAT)).astype(np.float32)
    bb = rng.normal(size=(1,)).astype(np.float32)
    got = kernel(xx, mk, mv, ww, bb)
    pow4 = (4 ** np.arange(D_FEAT)).astype(np.int64)
    mc = (mk.astype(np.int64) * pow4).sum(1)
    qc = (xx.astype(np.int64) * pow4).sum(1)
    last = {}
    for j, c in enumerate(mc):
        last[c] = j
    exp = np.where(
        np.isin(qc, mc),
        mv[[last.get(c, 0) for c in qc]],
        (xx @ ww.T + bb)[:, 0],
    )[:, None]
    err = np.abs(got - exp).max()
    print("max abs err vs numpy model:", err)



# revision 4
# speedup vs baseline: 1.3049x; 1.3049x over previous
"""Trainium2 Bass kernel for the exact-match memorizer lookup (v3).

Dense PE brute force, queries sharded 512/core, memory replicated.

Host prepares (pure layout / trivially-derived constants, all bf16-exact):
  ktrh [16, 32768] bf16: contraction-major augmented key matrix.
      Rows 0-7:  k_d (features, ints 0..3)
      Row  8:    |k|^2 (int <= 72)
      Row  9:    0.25            (ramp base, paired with x-side 1)
      Row 10:    -A * 2^-8       (A = (j mod 2048) >> 5, 6 bits)
      Row 11:    -B * 2^-13      (B = j mod 32, 5 bits)
      Row 12:    1.0             (|x|^2 carrier)
      Rows 13-15: 0
  xaugT [16, 512] f32: matching query-side columns:
      [-2 x_d (8), 1, 1, 1, 1, |x|^2, 0, 0, 0]
  mvpad [32768, 16] f32: mem_values broadcast to 64B rows (gather-friendly).

Per core, per query group g (128 queries) and tile t (2048 mem cols):
  one PSUM tile  ps[m, i] = |x|^2 - 2 x.k + |k|^2 + (2048 - i) * 2^-13
(exact in f32: all terms are multiples of 2^-13, total < 2^10).
A single fused DVE tensor_tensor_reduce (elementwise min of the two
1024-col halves + min-accumulate) yields per query the minimum over the
tile: matches give (2048 - i*) * 2^-13 <= 0.25 with i* the LAST matching
column (ramp strictly decreasing in i); non-matches give > 1.
Decode i* = 2048 - m * 8192, take max of (j_global + 1) * found over the
16 tiles, gather mvpad[jstar], select vs the linear fallback x @ w.T + b.
"""

import sys

if "/opt/trn_rl_repo" not in sys.path:
    sys.path.insert(0, "/opt/trn_rl_repo")

import numpy as np

import bass_rust
from concourse.bass import Bass, IndirectOffsetOnAxis
import concourse.tile as tile
from concourse import bass, mybir

N_QUERIES = 4096
N_MEM = 32768
D_FEAT = 8
N_CORES = 8
NQ = N_QUERIES // N_CORES  # 512 queries per core
QS = NQ // 128  # 4 query groups per core
KAUG = 16  # augmented contraction rows
TW = 2048  # mem cols per PSUM tile (4 banks)
NTILE = N_MEM // TW  # 16
MMN = 512  # moving-operand cols per matmul (PSUM out limited to one bank)
RS = 2.0 ** -13  # ramp scale

F32 = mybir.dt.float32
BF16 = mybir.dt.bfloat16
I32 = mybir.dt.int32
U8 = mybir.dt.uint8


def _patch_tile_drain():
    """This container's walrus accepts only one sync-wait per instruction;
    TileContext's teardown drain waits on every used semaphore at once.
    Split it into one drain per semaphore."""
    if getattr(tile.TileContext, "_drain_patched", False):
        return
    from concourse.tile import ScopedClock

    def _drain_and_barrier(self, tick_clock, wait_clock):
        gc = tick_clock.global_clock
        ticks = eval(repr(gc).replace("VectorClock(", "").rstrip(")"))
        for i, t in enumerate(ticks):
            if t <= 0:
                continue
            part = [t if j == i else 0 for j in range(len(ticks))]
            d = self.nc.sync.drain()
            wait_clock.add_sem_waits(
                d.ins, ScopedClock({None: bass_rust.VectorClock(part)})
            )
        self.nc.all_engine_barrier()
        assert self.sems is not None
        popped = self.nc._tile_sem_poison_stack.pop()
        assert popped is self._sem_poison
        self.nc.clear_and_free_semaphores(list(self.sems.allocated().values()))
        self.nc.all_engine_barrier()

    tile.TileContext._drain_and_barrier = _drain_and_barrier
    tile.TileContext._drain_patched = True


def _fix_multiwaits(bir_bytes: bytes) -> bytes:
    """Hoist extra sync-waits onto standalone EventSemaphore instructions
    inserted immediately before the offender (same engine => identical
    in-order blocking semantics)."""
    import json

    bir = json.loads(bir_bytes)
    for f in bir["functions"]:
        for blk in f["blocks"]:
            insts = blk["instructions"]
            out_insts = []
            changed = False
            for inst in insts:
                si = inst.get("sync_info")
                waits = si.get("on_wait", []) if si else []
                if len(waits) > 1:
                    changed = True
                    for k, wv in enumerate(waits[:-1]):
                        out_insts.append(
                            {
                                "debug": inst.get("debug", 0),
                                "engine": inst["engine"],
                                "ins": [],
                                "name": f"{inst['name']}-sw{k}",
                                "opcode": "EventSemaphore",
                                "outs": [],
                                "sync_info": {"on_update": [], "on_wait": [wv]},
                            }
                        )
                    si["on_wait"] = [waits[-1]]
                out_insts.append(inst)
            if changed:
                blk["instructions"] = out_insts
    return json.dumps(bir).encode()


def build_nc(debug: bool = False) -> Bass:
    _patch_tile_drain()
    nc = Bass()
    AX = mybir.AxisListType
    OP = mybir.AluOpType

    x = nc.dram_tensor("x", [NQ, D_FEAT], F32, kind="ExternalInput")
    ktrh = nc.dram_tensor("ktrh", [KAUG, N_MEM], BF16, kind="ExternalInput")
    xaugT = nc.dram_tensor("xaugT", [KAUG, NQ], F32, kind="ExternalInput")
    mvpad = nc.dram_tensor("mvpad", [N_MEM, 16], F32, kind="ExternalInput")
    w = nc.dram_tensor("w", [1, D_FEAT], F32, kind="ExternalInput")
    b = nc.dram_tensor("b", [1], F32, kind="ExternalInput")
    out = nc.dram_tensor("out", [NQ, 1], F32, kind="ExternalOutput")

    with tile.TileContext(nc) as tc:
        with (
            tc.tile_pool(name="sbuf", bufs=1) as pool,
            tc.tile_pool(name="work", bufs=4) as wpool,
            tc.tile_pool(name="psum", bufs=2, space="PSUM") as ppool,
        ):
            # ---- loads ------------------------------------------------------
            ktr_t = pool.tile([KAUG, N_MEM], BF16, tag="ktr")
            nc.sync.dma_start(out=ktr_t[:], in_=ktrh[:])

            xaT_f = pool.tile([KAUG, NQ], F32, tag="xaTf")
            nc.sync.dma_start(out=xaT_f[:], in_=xaugT[:])
            xaT = pool.tile([KAUG, NQ], BF16, tag="xaT")
            nc.vector.tensor_copy(out=xaT[:], in_=xaT_f[:])

            # x in layout B (q = g*128 + m): for the linear fallback
            xqb_t = pool.tile([128, QS * D_FEAT], F32, tag="xqb")
            nc.sync.dma_start(
                out=xqb_t[:].rearrange("p (g d) -> p g d", d=D_FEAT),
                in_=x[:].rearrange("(g m) d -> m g d", m=128),
            )
            xqb_v = xqb_t[:].rearrange("p (g d) -> p g d", d=D_FEAT)

            w_t = pool.tile([128, D_FEAT], F32, tag="wt")
            nc.sync.dma_start(out=w_t[:], in_=w[0:1, :].to_broadcast([128, D_FEAT]))
            b_t = pool.tile([128, 1], F32, tag="bt")
            nc.sync.dma_start(out=b_t[:], in_=b[None, :].to_broadcast([128, 1]))

            # ---- linear fallback linq[m, g] = x_q . w + b -------------------
            xw_t = pool.tile([128, QS * D_FEAT], F32, tag="xw")
            nc.vector.tensor_tensor(
                out=xw_t[:].rearrange("p (g d) -> p g d", d=D_FEAT),
                in0=xqb_v,
                in1=w_t[:, None, :].to_broadcast([128, QS, D_FEAT]),
                op=OP.mult,
            )
            linq_t = pool.tile([128, QS], F32, tag="linq")
            nc.vector.reduce_sum(
                out=linq_t[:],
                in_=xw_t[:].rearrange("p (g d) -> p g d", d=D_FEAT),
                axis=AX.X,
            )
            nc.vector.tensor_scalar_add(linq_t[:], linq_t[:], b_t[:, 0:1])

            # ---- main loop: matmul -> fused fold+min ------------------------
            # mins[m, g*16 + t] = min over tile t of group g
            mins_t = pool.tile([128, QS * NTILE], F32, tag="mins")

            val_t = pool.tile([128, QS], F32, tag="val")
            kk_t = pool.tile([128, QS], F32, tag="kk")
            ti_t = pool.tile([128, NTILE], I32, tag="ti")
            nc.gpsimd.iota(ti_t[:], [[1, NTILE]], base=0, channel_multiplier=0)
            tbase_t = pool.tile([128, NTILE], F32, tag="tbase")
            nc.vector.tensor_copy(out=tbase_t[:], in_=ti_t[:])
            nc.vector.tensor_scalar(
                out=tbase_t[:], in0=tbase_t[:], scalar1=float(TW), scalar2=1.0,
                op0=OP.mult, op1=OP.add,
            )  # t*2048 + 1

            for g in range(QS):
                lhsT = xaT[:, g * 128 : (g + 1) * 128]
                for t in range(NTILE):
                    ps = ppool.tile([128, TW], F32, tag="ps")
                    for k in range(TW // MMN):
                        nc.tensor.matmul(
                            out=ps[:, k * MMN : (k + 1) * MMN],
                            lhsT=lhsT,
                            rhs=ktr_t[:, t * TW + k * MMN : t * TW + (k + 1) * MMN],
                            start=True,
                            stop=True,
                        )
                    nc.vector.tensor_reduce(
                        out=mins_t[:, g * NTILE + t : g * NTILE + t + 1],
                        in_=ps[:, None, :],
                        axis=AX.X,
                        op=OP.min,
                    )

                # ---- per-group decode + gather ------------------------------
                gm = mins_t[:, g * NTILE : (g + 1) * NTILE]
                fo = wpool.tile([128, NTILE], F32, tag="fo")
                nc.vector.tensor_scalar(
                    out=fo[:], in0=gm, scalar1=0.5, scalar2=None, op0=OP.is_lt
                )
                ii = wpool.tile([128, NTILE], F32, tag="ii")
                nc.vector.tensor_scalar(
                    out=ii[:], in0=gm, scalar1=-8192.0, scalar2=float(TW),
                    op0=OP.mult, op1=OP.add,
                )  # i* = 2048 - m*8192
                nc.vector.tensor_tensor(out=ii[:], in0=ii[:], in1=tbase_t[:], op=OP.add)
                nc.vector.tensor_tensor(out=ii[:], in0=ii[:], in1=fo[:], op=OP.mult)
                nc.vector.reduce_max(
                    out=kk_t[:, g : g + 1], in_=ii[:], axis=AX.X
                )  # K = jstar + 1, 0 if not found

                rg = pool.tile([128, 1], F32, name=f"rgf{g}", tag=f"rgf{g}")
                nc.vector.tensor_scalar(
                    out=rg[:], in0=kk_t[:, g : g + 1], scalar1=-1.0, scalar2=0.0,
                    op0=OP.add, op1=OP.max,
                )
                rgi = pool.tile([128, 1], I32, name=f"rgi{g}", tag=f"rgi{g}")
                nc.vector.tensor_copy(out=rgi[:], in_=rg[:])
                vg = pool.tile([128, 16], F32, name=f"vg{g}", tag=f"vg{g}")
                nc.gpsimd.indirect_dma_start(
                    out=vg[:],
                    out_offset=None,
                    in_=mvpad[:],
                    in_offset=IndirectOffsetOnAxis(ap=rgi[:, 0:1], axis=0),
                )
                nc.vector.tensor_copy(out=val_t[:, g : g + 1], in_=vg[:, 0:1])

            # ---- select + store --------------------------------------------
            found_t = pool.tile([128, QS], F32, tag="found")
            nc.vector.tensor_scalar(
                out=found_t[:], in0=kk_t[:], scalar1=0.0, scalar2=None, op0=OP.is_gt
            )
            found_u8 = pool.tile([128, QS], U8, tag="found_u8")
            nc.vector.tensor_copy(out=found_u8[:], in_=found_t[:])
            res_t = pool.tile([128, QS], F32, tag="res")
            nc.vector.select(
                out=res_t[:],
                mask=found_u8[:],
                on_true=val_t[:],
                on_false=linq_t[:],
            )
            nc.sync.dma_start(
                out=out[:].rearrange("(g m) one -> m g one", m=128),
                in_=res_t[:, :, None],
            )

            if debug:
                taps = {
                    "d_mins": mins_t,
                    "d_kk": kk_t,
                    "d_linq": linq_t,
                    "d_val": val_t,
                }
                for name, t_ in taps.items():
                    shp = list(t_[:].shape)
                    dt_ = nc.dram_tensor(name, shp, F32, kind="ExternalOutput")
                    nc.sync.dma_start(out=dt_[:], in_=t_[:])

    return nc


_NC_CACHE: dict[str, Bass] = {}


def _get_nc() -> Bass:
    if "nc" not in _NC_CACHE:
        nc = build_nc()
        orig = nc.to_json_bytes
        nc.to_json_bytes = lambda: _fix_multiwaits(orig())
        _NC_CACHE["nc"] = nc
    return _NC_CACHE["nc"]


def _prep_keys(mem_keys: np.ndarray, mem_values: np.ndarray):
    """Host-side layout prep: contraction-major augmented key matrix (bf16,
    all values exactly representable) and 64B-padded value rows."""
    import ml_dtypes

    j = np.arange(N_MEM)
    i_loc = j % TW
    A = (i_loc >> 5).astype(np.float32)
    B = (i_loc & 31).astype(np.float32)
    ktr = np.zeros((KAUG, N_MEM), dtype=np.float32)
    ktr[0:D_FEAT, :] = mem_keys.T
    ktr[8, :] = (mem_keys * mem_keys).sum(axis=1)
    ktr[9, :] = 0.25
    ktr[10, :] = -A * 2.0 ** -8
    ktr[11, :] = -B * 2.0 ** -13
    ktr[12, :] = 1.0
    ktrh = ktr.astype(ml_dtypes.bfloat16)
    mvpad = np.ascontiguousarray(
        np.repeat(mem_values[:, None], 16, axis=1).astype(np.float32)
    )
    return ktrh, mvpad


def _prep_queries(xc: np.ndarray):
    """Host-side query-side augmented columns [16, 512] f32."""
    xa = np.zeros((KAUG, NQ), dtype=np.float32)
    xa[0:D_FEAT, :] = -2.0 * xc.T
    xa[8, :] = 1.0
    xa[9, :] = 1.0
    xa[10, :] = 1.0
    xa[11, :] = 1.0
    xa[12, :] = (xc * xc).sum(axis=1)
    return xa


def kernel(x, mem_keys, mem_values, w, b):
    from concourse.bass_utils import run_bass_kernel_spmd

    x = np.ascontiguousarray(np.asarray(x, dtype=np.float32))
    mem_keys = np.ascontiguousarray(np.asarray(mem_keys, dtype=np.float32))
    mem_values = np.ascontiguousarray(np.asarray(mem_values, dtype=np.float32))
    w = np.ascontiguousarray(np.asarray(w, dtype=np.float32))
    b = np.ascontiguousarray(np.asarray(b, dtype=np.float32))

    nc = _get_nc()
    ktrh, mvpad = _prep_keys(mem_keys, mem_values)
    in_maps = []
    for c in range(N_CORES):
        xc = x[c * NQ : (c + 1) * NQ]
        in_maps.append(
            {
                "x": xc,
                "ktrh": ktrh,
                "xaugT": _prep_queries(xc),
                "mvpad": mvpad,
                "w": w,
                "b": b,
            }
        )
    res = run_bass_kernel_spmd(nc, in_maps, core_ids=list(range(N_CORES)))
    return np.concatenate([r["out"] for r in res.results], axis=0)


if __name__ == "__main__":
    rng = np.random.default_rng(0)
    mk = rng.integers(0, 4, (N_MEM, D_FEAT)).astype(np.float32)
    xx = rng.integers(0, 4, (N_QUERIES, D_FEAT)).astype(np.float32)
    mv = rng.normal(size=N_MEM).astype(np.float32)
    ww = rng.normal(size=(1, D_FEAT)).astype(np.float32)
    bb = rng.normal(size=(1,)).astype(np.float32)
    got = kernel(xx, mk, mv, ww, bb)
    pow4 = (4 ** np.arange(D_FEAT)).astype(np.int64)
    mc = (mk.astype(np.int64) * pow4).sum(1)
    qc = (xx.astype(np.int64) * pow4).sum(1)
    last = {}
    for jj, c in enumerate(mc):
        last[c] = jj
    exp = np.where(
        np.isin(qc, mc),
        mv[[last.get(c, 0) for c in qc]],
        (xx @ ww.T + bb)[:, 0],
    )[:, None]
    err = np.abs(got - exp).max()
    print("max abs err vs numpy model:", err)
